# revision 3
# baseline (speedup 1.0000x reference)
"""Cluster-wise linear (MoE-style dense routing) Trainium2 kernel.

Computes out[t,o] = sum_c prob[t,c] * (x[t] @ W[c].T + b[c])[o] for
x (128,321,336) f32, prob (128,321,8), W (8,96,336), b (8,96).

Strategy: data-parallel over 8 NeuronCores (tokens = batch*n_vars split
evenly), 41 tiles of 128 tokens per core.

Host-side packing does the heavy lifting:
  - x is pre-transposed AND pre-cast to bf16 on the host into per-tile
    [K-partition, token] layout (the on-device PE transposes and the
    f32 DMA of the old version are gone; x DMA traffic halves).
  - a ones-row at K=336 folds the bias matmul in (W row 336 = b).

Device per tile (steady-state engine busy, cost-model):
  - Pool: one 96KB DMA of the pre-transposed x tile         (~300ns)
  - PE:   6 bf16 matmuls, Y[t, o*8+c] accum over 3 K-subtiles (960ns)
          -- this is the bf16 MAC roofline for the problem
  - ACT:  evict Y PSUM -> SBUF bf16                          (~780ns)
  - DVE:  prob-weighted cluster reduce, batched 4 tiles per
    instruction: tensor_tensor mult (2x mode) + 3 strided
    tensor_tensor add-tree levels                            (~905ns)
    (tensor_reduce has NO fast DVE mode -- 1x, 800ns/tile alone --
    which made the previous version DVE-bound; the add tree stays in
    the 2x path for all but the final 96-elem f32 level.)
"""

import numpy as np
import ml_dtypes

import concourse.bass as bass
import concourse.mybir as mybir
import concourse.tile as tile
from concourse.bass_utils import run_bass_kernel_spmd

N_CORES = 8
BSZ, N_VARS, IN_DIM, OUT_DIM, N_CLUSTER = 128, 321, 336, 96, 8
TOK = BSZ * N_VARS            # 41088
TPC = TOK // N_CORES          # 5136 tokens per core
P = 128
N_TILES = (TPC + P - 1) // P  # 41 (40 full + tail of 16; tail zero-padded)
TPAD = N_TILES * P            # 5248 padded tokens per core
KSUB = 3                      # K-subtiles: 336 data + 1 ones + 47 zeros = 384
IN_P = KSUB * P               # 384
KP = KSUB * P                 # per-tile free length of the xT tile
CO = OUT_DIM * N_CLUSTER      # 768, o-major: co = o*8 + c
BATCH = 4                     # stage-2 tiles per DVE instruction batch
N_BATCH = (N_TILES + BATCH - 1) // BATCH  # 11 (10 full + 1 of a single tile)


def split_multi_waits(nc):
    """This walrus build only supports one sync-wait per instruction; hoist
    extra waits onto same-engine nops inserted immediately before."""
    n_split = 0
    for fn in nc.m.functions:
        for bb in fn.blocks:
            insts = bb.instructions
            out = []
            changed = False
            for inst in insts:
                si = inst.sync_info
                if si is not None and si.on_wait and len(si.on_wait) > 1:
                    waits = list(si.on_wait)
                    del si.on_wait[1:]
                    si.on_wait[0] = waits[-1]
                    for w in waits[:-1]:
                        nop = mybir.InstNoOp(
                            name=f"{inst.name}-wsplit-{n_split}", ins=[], outs=[]
                        )
                        n_split += 1
                        nop.engine = inst.engine
                        nop.sync_info = mybir.SyncInfo(on_wait=[w], on_update=[])
                        out.append(nop)
                        changed = True
                out.append(inst)
            if changed:
                insts[:] = out
    return n_split


def build_nc(nrep: int = 1, loop_n: int = 1, bufs: int = 6, split_waits: bool = True,
             do_load=True, do_matmul=True, do_stage2=True):
    nc = bass.Bass()
    x_d = nc.dram_tensor(
        "xtp", [P, N_TILES * KP], mybir.dt.bfloat16, kind="ExternalInput"
    )
    p_d = nc.dram_tensor(
        "probp", [P, N_TILES * N_CLUSTER], mybir.dt.bfloat16, kind="ExternalInput"
    )
    w_d = nc.dram_tensor("wt", [IN_P, CO], mybir.dt.bfloat16, kind="ExternalInput")
    o_d = nc.dram_tensor("out", [TPAD, OUT_DIM], mybir.dt.float32, kind="ExternalOutput")
    # [p, q, o] view: dram row q*128+p
    o_v = o_d.rearrange("(q p) o -> p q o", p=P)

    dt = mybir.dt
    with tile.TileContext(nc) as tc:
        with (
            tc.tile_pool(name="const", bufs=1) as const,
            tc.tile_pool(name="work", bufs=1) as work,
            tc.tile_pool(name="psum", bufs=1, space="PSUM") as psum,
        ):
            # one-time loads
            wtb = const.tile([P, KSUB * CO], dt.bfloat16)
            wtb3 = wtb.rearrange("p (k n) -> p k n", k=KSUB)
            nc.gpsimd.dma_start(wtb3[:], w_d.rearrange("(k p) n -> p k n", p=P))
            pball = const.tile([P, N_TILES * N_CLUSTER], dt.bfloat16)
            nc.gpsimd.dma_start(pball[:], p_d[:])
            pb3 = pball.rearrange("p (j c) -> p j c", c=N_CLUSTER)

            # rings
            xt_ring = [work.tile([P, KP], dt.bfloat16, name=f"xt{i}") for i in range(bufs)]
            y_ring = [psum.tile([P, CO], dt.float32, name=f"yps{i}") for i in range(3)]
            ysb_ring = [
                work.tile([P, BATCH * CO], dt.bfloat16, name=f"ysb{i}") for i in range(2)
            ]
            z_ring = [work.tile([P, BATCH * CO], dt.bfloat16, name=f"z{i}") for i in range(2)]
            z1_ring = [
                work.tile([P, BATCH * OUT_DIM * 4], dt.bfloat16, name=f"z1_{i}")
                for i in range(2)
            ]
            z2_ring = [
                work.tile([P, BATCH * OUT_DIM * 2], dt.bfloat16, name=f"z2_{i}")
                for i in range(2)
            ]
            o_ring = [
                work.tile([P, BATCH * OUT_DIM], dt.float32, name=f"osb{i}") for i in range(2)
            ]
            if not do_load:
                for t in xt_ring:
                    nc.vector.memset(t[:], 0.0)
            if not do_matmul:
                for t in ysb_ring:
                    nc.vector.memset(t[:], 0.0)
            if not do_stage2:
                for t in o_ring:
                    nc.vector.memset(t[:], 0.0)

            def tile_mm(j: int):
                xt = xt_ring[j % bufs]
                if do_load:
                    nc.gpsimd.dma_start(xt[:], x_d[:, j * KP : (j + 1) * KP])
                xt3 = xt.rearrange("p (k t) -> p k t", k=KSUB)
                y = y_ring[j % 3]
                if do_matmul:
                    for k in range(KSUB):
                        for n0, n1 in ((0, 512), (512, CO)):
                            nc.tensor.matmul(
                                y[:, n0:n1],
                                xt3[:, k, :],
                                wtb3[:, k, n0:n1],
                                start=(k == 0),
                                stop=(k == KSUB - 1),
                            )
                    s = ysb_ring[(j // BATCH) % 2]
                    nc.scalar.copy(s[:, (j % BATCH) * CO : (j % BATCH + 1) * CO], y[:])

            def batch_stage2(b: int):
                nt = min(BATCH, N_TILES - b * BATCH)
                if do_stage2:
                    yv = ysb_ring[b % 2].rearrange(
                        "p (j o c) -> p j o c", o=OUT_DIM, c=N_CLUSTER
                    )[:, 0:nt]
                    pbc = (
                        pb3[:, b * BATCH : b * BATCH + nt, :]
                        .unsqueeze(2)
                        .broadcast_to([P, nt, OUT_DIM, N_CLUSTER])
                    )
                    zv = z_ring[b % 2].rearrange(
                        "p (j o c) -> p j o c", o=OUT_DIM, c=N_CLUSTER
                    )[:, 0:nt]
                    nc.vector.tensor_tensor(zv, yv, pbc, mybir.AluOpType.mult)
                    z1v = z1_ring[b % 2].rearrange(
                        "p (j o c) -> p j o c", o=OUT_DIM, c=4
                    )[:, 0:nt]
                    nc.vector.tensor_tensor(
                        z1v, zv[:, :, :, 0:4], zv[:, :, :, 4:8], mybir.AluOpType.add
                    )
                    z2v = z2_ring[b % 2].rearrange(
                        "p (j o c) -> p j o c", o=OUT_DIM, c=2
                    )[:, 0:nt]
                    nc.vector.tensor_tensor(
                        z2v, z1v[:, :, :, 0:2], z1v[:, :, :, 2:4], mybir.AluOpType.add
                    )
                    ov = o_ring[b % 2].rearrange("p (j o) -> p j o", o=OUT_DIM)[:, 0:nt]
                    nc.vector.tensor_tensor(
                        ov, z2v[:, :, :, 0], z2v[:, :, :, 1], mybir.AluOpType.add
                    )
                else:
                    ov = o_ring[b % 2].rearrange("p (j o) -> p j o", o=OUT_DIM)[:, 0:nt]
                nc.gpsimd.dma_start(o_v[:, b * BATCH : b * BATCH + nt, :], ov)

            def sweep(_iv=None):
                for j in range(N_TILES):
                    tile_mm(j)
                    if j % BATCH == BATCH - 1 or j == N_TILES - 1:
                        batch_stage2(j // BATCH)

            if loop_n > 1:
                with tc.For_i(0, loop_n):
                    for _ in range(nrep):
                        sweep()
            else:
                for _ in range(nrep):
                    sweep()

    if split_waits:
        split_multi_waits(nc)
    return nc


def pack_inputs(x, prob, W, b):
    """Host-side packing. Returns per-core input maps."""
    x = np.asarray(x, dtype=np.float32).reshape(TOK, IN_DIM)
    prob = np.asarray(prob, dtype=np.float32).reshape(TOK, N_CLUSTER)
    W = np.asarray(W, dtype=np.float32)
    b = np.asarray(b, dtype=np.float32)

    # weights: wt[i, o*8+c] = W[c,o,i]; bias row at i=336; zeros to IN_P
    wt = np.zeros((IN_P, CO), dtype=np.float32)
    wt[:IN_DIM] = W.transpose(2, 1, 0).reshape(IN_DIM, CO)
    wt[IN_DIM] = b.T.reshape(CO)
    wt16 = np.ascontiguousarray(wt.astype(ml_dtypes.bfloat16))

    in_maps = []
    for c in range(N_CORES):
        xs = x[c * TPC : (c + 1) * TPC]
        # padded token-major [TPAD, IN_P] with ones column, then transpose to
        # [p, (j, k, t)] where element = xpad[j*128 + t, k*128 + p]
        xp = np.zeros((TPAD, IN_P), dtype=np.float32)
        xp[:TPC, :IN_DIM] = xs
        xp[:, IN_DIM] = 1.0
        a = xp.reshape(N_TILES, P, KSUB, P)          # [j, t, k, p]
        xtp = np.ascontiguousarray(
            a.transpose(3, 0, 2, 1).astype(ml_dtypes.bfloat16)
        ).reshape(P, N_TILES * KP)

        ps = prob[c * TPC : (c + 1) * TPC]
        pp = np.zeros((TPAD, N_CLUSTER), dtype=np.float32)
        pp[:TPC] = ps
        # (j, t, c) -> (t, j, c)
        pp = pp.reshape(N_TILES, P, N_CLUSTER).transpose(1, 0, 2)
        pp16 = np.ascontiguousarray(
            pp.astype(ml_dtypes.bfloat16).reshape(P, N_TILES * N_CLUSTER)
        )
        in_maps.append({"xtp": xtp, "probp": pp16, "wt": wt16})
    return in_maps


_cached = {}


def kernel(x, prob, W, b):
    key = "main"
    if key not in _cached:
        _cached[key] = build_nc(nrep=1)
    nc = _cached[key]
    in_maps = pack_inputs(x, prob, W, b)
    res = run_bass_kernel_spmd(nc, in_maps, list(range(N_CORES)))
    outs = [res.results[c]["out"][:TPC] for c in range(N_CORES)]
    out = np.concatenate(outs, axis=0).reshape(BSZ, N_VARS, OUT_DIM)
    return out.astype(np.float32)


if __name__ == "__main__":
    rng = np.random.default_rng(0)
    x = rng.standard_normal((BSZ, N_VARS, IN_DIM)).astype(np.float32)
    prob = rng.random((BSZ, N_VARS, N_CLUSTER)).astype(np.float32)
    W = (rng.standard_normal((N_CLUSTER, OUT_DIM, IN_DIM)) / 18.3).astype(np.float32)
    b = rng.standard_normal((N_CLUSTER, OUT_DIM)).astype(np.float32) / 18.3
    out = kernel(x, prob, W, b)
    ref = np.einsum("ti,coi,tc->to", x.reshape(TOK, IN_DIM), W,
                    prob.reshape(TOK, N_CLUSTER)) + prob.reshape(TOK, N_CLUSTER) @ b
    ref = ref.reshape(BSZ, N_VARS, OUT_DIM)
    err = np.linalg.norm(out - ref) / np.linalg.norm(ref)
    print("rel_l2:", err)


# revision 6
# speedup vs baseline: 1.2900x; 1.2900x over previous
"""Cluster-wise linear (MoE-style dense routing) Trainium2 kernel.

Computes out[t,o] = sum_c prob[t,c] * (x[t] @ W[c].T + b[c])[o] for
x (128,321,336) f32, prob (128,321,8), W (8,96,336), b (8,96).

Strategy: data-parallel over 8 NeuronCores (tokens = batch*n_vars split
evenly), 41 tiles of 128 tokens per core.

Host-side packing does the heavy lifting:
  - x is pre-transposed AND pre-cast to bf16 on the host into per-tile
    [K-partition, token] layout (the on-device PE transposes and the
    f32 DMA of the old version are gone; x DMA traffic halves).
  - a ones-row at K=336 folds the bias matmul in (W row 336 = b).

Device per tile (steady-state engine busy, cost-model):
  - Pool: one 96KB DMA of the pre-transposed x tile         (~300ns)
  - PE:   6 bf16 matmuls, Y[t, o*8+c] accum over 3 K-subtiles (960ns)
          -- this is the bf16 MAC roofline for the problem
  - ACT:  evict Y PSUM -> SBUF bf16                          (~780ns)
  - DVE:  prob-weighted cluster reduce, batched 4 tiles per
    instruction: tensor_tensor mult (2x mode) + 3 strided
    tensor_tensor add-tree levels                            (~905ns)
    (tensor_reduce has NO fast DVE mode -- 1x, 800ns/tile alone --
    which made the previous version DVE-bound; the add tree stays in
    the 2x path for all but the final 96-elem f32 level.)

PSUM ring is 4 deep (all 8 banks); the preamble loads tile 0's x and
then the weights per K-subtile so the first matmul starts ~1.4us in.
Cost model (CoreSim, tracked HW within ~11% on the old version):
single-shot 50.9us, steady-state 39.5us/sweep per core = 99% of the
bf16 PE MAC roofline (2304 cycles/tile x 41 tiles @ 2.4GHz).  The old
DVE-bound version measured 71.5us on HW.  HW rel err 4.4e-3.
"""

import numpy as np
import ml_dtypes

import concourse.bass as bass
import concourse.mybir as mybir
import concourse.tile as tile
from concourse.bass_utils import run_bass_kernel_spmd

N_CORES = 8
BSZ, N_VARS, IN_DIM, OUT_DIM, N_CLUSTER = 128, 321, 336, 96, 8
TOK = BSZ * N_VARS            # 41088
TPC = TOK // N_CORES          # 5136 tokens per core
P = 128
N_TILES = (TPC + P - 1) // P  # 41 (40 full + tail of 16; tail zero-padded)
TPAD = N_TILES * P            # 5248 padded tokens per core
KSUB = 3                      # K-subtiles: 336 data + 1 ones + 47 zeros = 384
IN_P = KSUB * P               # 384
KP = KSUB * P                 # per-tile free length of the xT tile
CO = OUT_DIM * N_CLUSTER      # 768, o-major: co = o*8 + c
BATCH = 4                     # stage-2 tiles per DVE instruction batch
N_BATCH = (N_TILES + BATCH - 1) // BATCH  # 11 (10 full + 1 of a single tile)


def split_multi_waits(nc):
    """This walrus build only supports one sync-wait per instruction; hoist
    extra waits onto same-engine nops inserted immediately before."""
    n_split = 0
    for fn in nc.m.functions:
        for bb in fn.blocks:
            insts = bb.instructions
            out = []
            changed = False
            for inst in insts:
                si = inst.sync_info
                if si is not None and si.on_wait and len(si.on_wait) > 1:
                    waits = list(si.on_wait)
                    del si.on_wait[1:]
                    si.on_wait[0] = waits[-1]
                    for w in waits[:-1]:
                        nop = mybir.InstNoOp(
                            name=f"{inst.name}-wsplit-{n_split}", ins=[], outs=[]
                        )
                        n_split += 1
                        nop.engine = inst.engine
                        nop.sync_info = mybir.SyncInfo(on_wait=[w], on_update=[])
                        out.append(nop)
                        changed = True
                out.append(inst)
            if changed:
                insts[:] = out
    return n_split


def build_nc(nrep: int = 1, loop_n: int = 1, bufs: int = 6, split_waits: bool = True,
             do_load=True, do_matmul=True, do_stage2=True):
    nc = bass.Bass()
    x_d = nc.dram_tensor(
        "xtp", [P, N_TILES * KP], mybir.dt.bfloat16, kind="ExternalInput"
    )
    p_d = nc.dram_tensor(
        "probp", [P, N_TILES * N_CLUSTER], mybir.dt.bfloat16, kind="ExternalInput"
    )
    w_d = nc.dram_tensor("wt", [IN_P, CO], mybir.dt.bfloat16, kind="ExternalInput")
    o_d = nc.dram_tensor("out", [TPAD, OUT_DIM], mybir.dt.float32, kind="ExternalOutput")
    # [p, q, o] view: dram row q*128+p
    o_v = o_d.rearrange("(q p) o -> p q o", p=P)

    dt = mybir.dt
    with tile.TileContext(nc) as tc:
        with (
            tc.tile_pool(name="const", bufs=1) as const,
            tc.tile_pool(name="work", bufs=1) as work,
            tc.tile_pool(name="psum", bufs=1, space="PSUM") as psum,
        ):
            # rings
            xt_ring = [work.tile([P, KP], dt.bfloat16, name=f"xt{i}") for i in range(bufs)]
            y_ring = [psum.tile([P, CO], dt.float32, name=f"yps{i}") for i in range(4)]

            # one-time loads: tile 0's x first, then weights per K-subtile,
            # so the first matmul waits ~1.4us of preamble DMA, not ~4us
            wtb = const.tile([P, KSUB * CO], dt.bfloat16)
            wtb3 = wtb.rearrange("p (k n) -> p k n", k=KSUB)
            w_v = w_d.rearrange("(k p) n -> p k n", p=P)
            if do_load:
                nc.gpsimd.dma_start(xt_ring[0][:], x_d[:, 0:KP])
            for k in range(KSUB):
                nc.gpsimd.dma_start(wtb3[:, k, :], w_v[:, k, :])
            pball = const.tile([P, N_TILES * N_CLUSTER], dt.bfloat16)
            nc.gpsimd.dma_start(pball[:], p_d[:])
            pb3 = pball.rearrange("p (j c) -> p j c", c=N_CLUSTER)
            ysb_ring = [
                work.tile([P, BATCH * CO], dt.bfloat16, name=f"ysb{i}") for i in range(2)
            ]
            z_ring = [work.tile([P, BATCH * CO], dt.bfloat16, name=f"z{i}") for i in range(2)]
            z1_ring = [
                work.tile([P, BATCH * OUT_DIM * 4], dt.bfloat16, name=f"z1_{i}")
                for i in range(2)
            ]
            z2_ring = [
                work.tile([P, BATCH * OUT_DIM * 2], dt.bfloat16, name=f"z2_{i}")
                for i in range(2)
            ]
            o_ring = [
                work.tile([P, BATCH * OUT_DIM], dt.float32, name=f"osb{i}") for i in range(2)
            ]
            if not do_load:
                for t in xt_ring:
                    nc.vector.memset(t[:], 0.0)
            if not do_matmul:
                for t in ysb_ring:
                    nc.vector.memset(t[:], 0.0)
            if not do_stage2:
                for t in o_ring:
                    nc.vector.memset(t[:], 0.0)

            first_sweep = [True]

            def tile_mm(j: int):
                xt = xt_ring[j % bufs]
                if do_load and not (j == 0 and first_sweep[0]):
                    nc.gpsimd.dma_start(xt[:], x_d[:, j * KP : (j + 1) * KP])
                xt3 = xt.rearrange("p (k t) -> p k t", k=KSUB)
                y = y_ring[j % 4]
                if do_matmul:
                    for k in range(KSUB):
                        for n0, n1 in ((0, 512), (512, CO)):
                            nc.tensor.matmul(
                                y[:, n0:n1],
                                xt3[:, k, :],
                                wtb3[:, k, n0:n1],
                                start=(k == 0),
                                stop=(k == KSUB - 1),
                            )
                    s = ysb_ring[(j // BATCH) % 2]
                    nc.scalar.copy(s[:, (j % BATCH) * CO : (j % BATCH + 1) * CO], y[:])

            def batch_stage2(b: int):
                nt = min(BATCH, N_TILES - b * BATCH)
                if do_stage2:
                    yv = ysb_ring[b % 2].rearrange(
                        "p (j o c) -> p j o c", o=OUT_DIM, c=N_CLUSTER
                    )[:, 0:nt]
                    pbc = (
                        pb3[:, b * BATCH : b * BATCH + nt, :]
                        .unsqueeze(2)
                        .broadcast_to([P, nt, OUT_DIM, N_CLUSTER])
                    )
                    zv = z_ring[b % 2].rearrange(
                        "p (j o c) -> p j o c", o=OUT_DIM, c=N_CLUSTER
                    )[:, 0:nt]
                    nc.vector.tensor_tensor(zv, yv, pbc, mybir.AluOpType.mult)
                    z1v = z1_ring[b % 2].rearrange(
                        "p (j o c) -> p j o c", o=OUT_DIM, c=4
                    )[:, 0:nt]
                    nc.vector.tensor_tensor(
                        z1v, zv[:, :, :, 0:4], zv[:, :, :, 4:8], mybir.AluOpType.add
                    )
                    z2v = z2_ring[b % 2].rearrange(
                        "p (j o c) -> p j o c", o=OUT_DIM, c=2
                    )[:, 0:nt]
                    nc.vector.tensor_tensor(
                        z2v, z1v[:, :, :, 0:2], z1v[:, :, :, 2:4], mybir.AluOpType.add
                    )
                    ov = o_ring[b % 2].rearrange("p (j o) -> p j o", o=OUT_DIM)[:, 0:nt]
                    nc.vector.tensor_tensor(
                        ov, z2v[:, :, :, 0], z2v[:, :, :, 1], mybir.AluOpType.add
                    )
                else:
                    ov = o_ring[b % 2].rearrange("p (j o) -> p j o", o=OUT_DIM)[:, 0:nt]
                nc.gpsimd.dma_start(o_v[:, b * BATCH : b * BATCH + nt, :], ov)

            def sweep(_iv=None):
                for j in range(N_TILES):
                    tile_mm(j)
                    if j % BATCH == BATCH - 1 or j == N_TILES - 1:
                        batch_stage2(j // BATCH)
                first_sweep[0] = False

            if loop_n > 1:
                with tc.For_i(0, loop_n):
                    for _ in range(nrep):
                        sweep()
            else:
                for _ in range(nrep):
                    sweep()

    if split_waits:
        split_multi_waits(nc)
    return nc


def pack_inputs(x, prob, W, b):
    """Host-side packing. Returns per-core input maps."""
    x = np.asarray(x, dtype=np.float32).reshape(TOK, IN_DIM)
    prob = np.asarray(prob, dtype=np.float32).reshape(TOK, N_CLUSTER)
    W = np.asarray(W, dtype=np.float32)
    b = np.asarray(b, dtype=np.float32)

    # weights: wt[i, o*8+c] = W[c,o,i]; bias row at i=336; zeros to IN_P
    wt = np.zeros((IN_P, CO), dtype=np.float32)
    wt[:IN_DIM] = W.transpose(2, 1, 0).reshape(IN_DIM, CO)
    wt[IN_DIM] = b.T.reshape(CO)
    wt16 = np.ascontiguousarray(wt.astype(ml_dtypes.bfloat16))

    in_maps = []
    for c in range(N_CORES):
        xs = x[c * TPC : (c + 1) * TPC]
        # padded token-major [TPAD, IN_P] with ones column, then transpose to
        # [p, (j, k, t)] where element = xpad[j*128 + t, k*128 + p]
        xp = np.zeros((TPAD, IN_P), dtype=np.float32)
        xp[:TPC, :IN_DIM] = xs
        xp[:, IN_DIM] = 1.0
        a = xp.reshape(N_TILES, P, KSUB, P)          # [j, t, k, p]
        xtp = np.ascontiguousarray(
            a.transpose(3, 0, 2, 1).astype(ml_dtypes.bfloat16)
        ).reshape(P, N_TILES * KP)

        ps = prob[c * TPC : (c + 1) * TPC]
        pp = np.zeros((TPAD, N_CLUSTER), dtype=np.float32)
        pp[:TPC] = ps
        # (j, t, c) -> (t, j, c)
        pp = pp.reshape(N_TILES, P, N_CLUSTER).transpose(1, 0, 2)
        pp16 = np.ascontiguousarray(
            pp.astype(ml_dtypes.bfloat16).reshape(P, N_TILES * N_CLUSTER)
        )
        in_maps.append({"xtp": xtp, "probp": pp16, "wt": wt16})
    return in_maps


_cached = {}


def kernel(x, prob, W, b):
    key = "main"
    if key not in _cached:
        _cached[key] = build_nc(nrep=1)
    nc = _cached[key]
    in_maps = pack_inputs(x, prob, W, b)
    res = run_bass_kernel_spmd(nc, in_maps, list(range(N_CORES)))
    outs = [res.results[c]["out"][:TPC] for c in range(N_CORES)]
    out = np.concatenate(outs, axis=0).reshape(BSZ, N_VARS, OUT_DIM)
    return out.astype(np.float32)


if __name__ == "__main__":
    rng = np.random.default_rng(0)
    x = rng.standard_normal((BSZ, N_VARS, IN_DIM)).astype(np.float32)
    prob = rng.random((BSZ, N_VARS, N_CLUSTER)).astype(np.float32)
    W = (rng.standard_normal((N_CLUSTER, OUT_DIM, IN_DIM)) / 18.3).astype(np.float32)
    b = rng.standard_normal((N_CLUSTER, OUT_DIM)).astype(np.float32) / 18.3
    out = kernel(x, prob, W, b)
    ref = np.einsum("ti,coi,tc->to", x.reshape(TOK, IN_DIM), W,
                    prob.reshape(TOK, N_CLUSTER)) + prob.reshape(TOK, N_CLUSTER) @ b
    ref = ref.reshape(BSZ, N_VARS, OUT_DIM)
    err = np.linalg.norm(out - ref) / np.linalg.norm(ref)
    print("rel_l2:", err)
